# revision 1
# baseline (speedup 1.0000x reference)
"""Trainium2 Bass kernel for BalancedConformationalConsistencyLoss.

Strategy (segment/fragment parallelism, 8 cores):
  * Host sorts nodes by fragment; 32 fragments per core (snake-deal by size),
    bin-packed into 9 strips of 128 slots so no fragment straddles a strip.
  * Device (per core, SPMD): encoder MLPs in feature-major (D on partitions)
    layout; per-node L2 norms via ones-matmul + DRAM bounce; normalized
    features (bf16); per-strip 128x128 gram blocks on the tensor engine;
    masked pair reductions via scalar_tensor_tensor/activation accumulators;
    sums via one-hot matmuls; per-core partial loss -> [1,3] output.
  * Host combines 8 partials (sum + the two global Frobenius sqrts).

All pairwise math uses the identity  sum_pm (S-t)^2 = A - 2tB + t^2*npairs
with A = sum pm*S^2, B = sum pm*S, so no (S-t) intermediates are built.
"""
import numpy as np
from contextlib import ExitStack

# ---------------- problem constants (hardcoded per contract) ----------------
N, D, NF = 8192, 256, 256
R = 0.3
BRICS = 0.4
AW = 0.6
VW = 0.2
CF = 0.1
NCORES = 8
NFC = NF // NCORES          # 32 fragments per core
P = 128                     # strip height
NSTRIP = 9                  # strips per core
M = NSTRIP * P              # 1152 padded slots per core
CH = 384                    # encoder column chunk
NCH = M // CH
LN2 = np.float32(np.log(2.0))

# Q column indices
(QB_SH, QA_SH, QA_UN, QC_UN, QB_DIR, QA_DIR,
 QSOFF, QVOFF, QNSQ_SH, QNSQ_UN, QNSQ_V, QJUNK) = range(12)
NQ = 12


# ============================ host-side prep ================================

def _assign_fragments(fid):
    counts = np.bincount(fid, minlength=NF)
    order = np.argsort(-counts, kind="stable")
    core_frags = [[] for _ in range(NCORES)]
    for i, f in enumerate(order):
        r = i // NCORES
        c = i % NCORES if r % 2 == 0 else NCORES - 1 - (i % NCORES)
        core_frags[c].append(int(f))
    layout = []
    for c in range(NCORES):
        frags = sorted(core_frags[c], key=lambda f: -counts[f])
        strips = [[] for _ in range(NSTRIP)]
        fill = np.zeros(NSTRIP, dtype=int)
        for f in frags:
            for s in range(NSTRIP):
                if fill[s] + counts[f] <= P:
                    strips[s].append(f)
                    fill[s] += counts[f]
                    break
            else:
                raise AssertionError(f"core {c}: fragment {f} does not fit")
        layout.append(strips)
    return layout, counts


def _build_core_meta(fid, atom_types, layout, counts):
    nodes_of = {f: np.nonzero(fid == f)[0] for f in range(NF)}
    metas = []
    for c in range(NCORES):
        slot_node = -np.ones(M, dtype=np.int64)
        slot_frag = -np.ones(M, dtype=np.int64)
        frag_global = []
        fl = 0
        for b in range(NSTRIP):
            pos = b * P
            for f in layout[c][b]:
                nn = nodes_of[f]
                slot_node[pos:pos + len(nn)] = nn
                slot_frag[pos:pos + len(nn)] = fl
                frag_global.append(f)
                pos += len(nn)
                fl += 1
        assert fl == NFC
        frag_global = np.array(frag_global, dtype=np.int64)
        real = slot_node >= 0

        pm3 = np.zeros((NSTRIP, P, 3 * P), dtype=np.float32)
        tm = np.zeros((NSTRIP, P, P), dtype=np.float32)
        amat = np.zeros((NSTRIP, P, NFC), dtype=np.float32)
        t2pm_frag = np.zeros(NFC, dtype=np.float32)
        for b in range(NSTRIP):
            sf = slot_frag[b * P:(b + 1) * P]
            sn = slot_node[b * P:(b + 1) * P]
            rr = sf >= 0
            same = (sf[:, None] == sf[None, :]) & rr[:, None] & rr[None, :]
            upper = np.triu(np.ones((P, P), dtype=bool), k=1)
            pmb = (same & upper).astype(np.float32)
            pm3[b] = np.tile(pmb, (1, 3))
            at = np.where(rr, atom_types[np.where(rr, sn, 0)], -1)
            tgt = np.where(at[:, None] == at[None, :], 0.3, 0.1).astype(np.float32)
            tm[b] = tgt * pmb
            for p in range(P):
                if rr[p]:
                    amat[b, p, sf[p]] = 1.0
            for fl_ in np.unique(sf[rr]):
                sel = sf == fl_
                t2pm_frag[fl_] += float(((tgt * pmb) ** 2)[np.ix_(sel, sel)].sum())

        cnt = counts[frag_global].astype(np.float32)
        valid = (cnt >= 2.0).astype(np.float32)
        pairs = cnt * (cnt - 1.0) * 0.5
        safe_c = np.maximum(cnt, 1.0)
        safe_p = np.maximum(pairs, 1.0)

        cvec = np.zeros((NFC, NQ), dtype=np.float32)
        t_sh = np.float32(BRICS)
        t_dir = np.float32(BRICS - 0.2)
        cvec[:, QB_SH] = -2.0 * t_sh * AW / safe_p
        cvec[:, QA_SH] = AW / safe_p
        cvec[:, QA_UN] = (1.0 - AW) / safe_p
        cvec[:, QC_UN] = -2.0 * (1.0 - AW) / safe_p
        cvec[:, QB_DIR] = -2.0 * t_dir * VW / (3.0 * safe_p)
        cvec[:, QA_DIR] = VW / (3.0 * safe_p)
        cvec[:, QSOFF] = -2.0 * AW / (safe_c * safe_c)
        cvec[:, QVOFF] = -2.0 * VW / (safe_c * safe_c)
        cvec[:, QNSQ_SH] = AW * (1.0 / safe_c - 1.0 / (safe_c * safe_c))
        cvec[:, QNSQ_V] = VW * (1.0 / safe_c - 1.0 / (safe_c * safe_c))
        cvec[:, QJUNK] = (AW * t_sh * t_sh * pairs / safe_p
                          + (1.0 - AW) * t2pm_frag / safe_p
                          + VW * t_dir * t_dir * pairs / safe_p)
        cvec *= valid[:, None]

        metas.append(dict(slot_node=slot_node, real=real, pm3=pm3, tm=tm,
                          amat=amat, cvec=cvec))
    return metas


def _shard_inputs(inputs):
    fid = np.asarray(inputs["fragment_ids"]).astype(np.int64)
    at = np.asarray(inputs["atom_types"]).astype(np.int64)
    layout, counts = _assign_fragments(fid)
    metas = _build_core_meta(fid, at, layout, counts)

    W1 = np.asarray(inputs["W1"], np.float32)
    W2 = np.asarray(inputs["W2"], np.float32)
    Wd1 = np.asarray(inputs["Wd1"], np.float32)
    Wd2 = np.asarray(inputs["Wd2"], np.float32)
    Wv1 = np.asarray(inputs["Wv1"], np.float32)
    Wv2 = np.asarray(inputs["Wv2"], np.float32)
    w1c = np.ascontiguousarray(np.concatenate([R * W1, (1.0 - R) * W1], axis=0), np.float32)
    wv1c = np.ascontiguousarray(np.concatenate([R * Wv1, (1.0 - R) * Wv1], axis=0), np.float32)
    b2p = (np.asarray(inputs["b2"], np.float32) - LN2 * W2.sum(axis=0)).astype(np.float32)
    bd2p = (np.asarray(inputs["bd2"], np.float32) - LN2 * Wd2.sum(axis=0)).astype(np.float32)
    bv2p = (-LN2 * Wv2.sum(axis=0)).astype(np.float32)

    ss = np.asarray(inputs["scalar_short"], np.float32)
    sl = np.asarray(inputs["scalar_long"], np.float32)
    vs = np.asarray(inputs["vector_short"], np.float32)
    vl = np.asarray(inputs["vector_long"], np.float32)

    in_maps = []
    for c in range(NCORES):
        m = metas[c]
        idx = np.where(m["real"], m["slot_node"], 0)
        rmask2 = m["real"][:, None]

        def take2(x):
            g = x[idx] * rmask2
            return np.ascontiguousarray(g.T, dtype=np.float32)

        def take3(x):
            g = x[idx] * m["real"][:, None, None]
            return np.ascontiguousarray(
                g.transpose(1, 2, 0).reshape(3 * D, M), dtype=np.float32)

        in_maps.append({
            "x_s": take2(ss), "x_l": take2(sl),
            "v_s": take3(vs), "v_l": take3(vl),
            "w1c": w1c, "w2": W2, "wd1": Wd1, "wd2": Wd2,
            "wv1c": wv1c, "wv2": Wv2,
            "b1": np.asarray(inputs["b1"], np.float32).reshape(D, 1),
            "b2p": b2p.reshape(D, 1),
            "bd1": np.asarray(inputs["bd1"], np.float32).reshape(2 * D, 1),
            "bd2p": bd2p.reshape(2 * D, 1),
            "bv2p": bv2p.reshape(D, 1),
            "pm3": m["pm3"], "tm": m["tm"],
            "amat": m["amat"], "cvec": m["cvec"],
            "ident3": np.ascontiguousarray(np.tile(np.eye(P, dtype=np.float32), (1, 3))),
        })
    n_valid = float((counts >= 2).sum())
    return in_maps, n_valid


def _combine(fins, n_valid):
    loss = float(sum(float(f[0]) for f in fins))
    ssq_sh = float(sum(float(f[1]) for f in fins))
    ssq_un = float(sum(float(f[2]) for f in fins))
    l2 = 0.01 * (np.sqrt(ssq_sh) + np.sqrt(ssq_un))
    if n_valid > 0:
        return np.float32(CF * (loss + n_valid * l2) / max(n_valid, 1.0))
    return np.float32(0.0)


# ============================ device program ================================

_NC_CACHE = {}


def build_nc():
    import os
    PH = int(os.environ.get("BCCL_PHASES", "9"))
    if "nc" in _NC_CACHE:
        return _NC_CACHE["nc"]
    import concourse.bass as bass
    import concourse.bacc as bacc
    import concourse.mybir as mybir
    import concourse.tile as tile

    F32 = mybir.dt.float32
    BF16 = mybir.dt.bfloat16
    AF = mybir.ActivationFunctionType
    ALU = mybir.AluOpType

    nc = bacc.Bacc("TRN2", target_bir_lowering=False, debug=False)

    d = {}
    for name, shape in [
        ("x_s", [D, M]), ("x_l", [D, M]), ("v_s", [3 * D, M]), ("v_l", [3 * D, M]),
        ("w1c", [2 * D, D]), ("w2", [D, D]), ("wd1", [D, 2 * D]), ("wd2", [2 * D, 2 * D]),
        ("wv1c", [2 * D, D]), ("wv2", [D, D]),
        ("b1", [D, 1]), ("b2p", [D, 1]), ("bd1", [2 * D, 1]), ("bd2p", [2 * D, 1]),
        ("bv2p", [D, 1]),
        ("pm3", [NSTRIP, P, 3 * P]), ("tm", [NSTRIP, P, P]),
        ("amat", [NSTRIP, P, NFC]), ("cvec", [NFC, NQ]),
        ("ident3", [P, 3 * P]),
    ]:
        d[name] = nc.dram_tensor(name, shape, F32, kind="ExternalInput").ap()
    d_out = nc.dram_tensor("out", [1, 3], F32, kind="ExternalOutput").ap()
    d_inv = nc.dram_tensor("inv_scratch", [5, M], F32).ap()

    with tile.TileContext(nc) as tc, ExitStack() as ctx:
        wpool = ctx.enter_context(tc.tile_pool(name="w", bufs=1))
        feat = ctx.enter_context(tc.tile_pool(name="feat", bufs=1))
        xin = ctx.enter_context(tc.tile_pool(name="xin", bufs=2))
        small = ctx.enter_context(tc.tile_pool(name="small", bufs=1))
        rowp = ctx.enter_context(tc.tile_pool(name="rowp", bufs=2))
        junkp = ctx.enter_context(tc.tile_pool(name="junk", bufs=3))
        maskp = ctx.enter_context(tc.tile_pool(name="mask", bufs=2))
        psp = ctx.enter_context(tc.tile_pool(name="ps", bufs=6, space="PSUM"))
        pseg = ctx.enter_context(tc.tile_pool(name="pseg", bufs=1, space="PSUM"))

        # ---- constants / weights ----
        def load_w(name, kt, cols):
            ts_ = []
            for k in range(kt):
                w = wpool.tile([P, cols], F32, tag=f"{name}{k}", name=f"{name}{k}")
                nc.sync.dma_start(out=w, in_=d[name][k * P:(k + 1) * P, :])
                ts_.append(w)
            return ts_

        w1c_t = load_w("w1c", 4, D)
        w2_t = load_w("w2", 2, D)
        wd1_t = load_w("wd1", 2, 2 * D)
        wd2_t = load_w("wd2", 4, 2 * D)
        wv1c_t = load_w("wv1c", 4, D)
        wv2_t = load_w("wv2", 2, D)

        def load_bias(name, mt):
            b = wpool.tile([P, mt], F32, tag=f"b_{name}", name=f"b_{name}")
            nc.sync.dma_start(out=b, in_=d[name].rearrange("(m p) o -> p (m o)", p=P))
            return b

        b1_sb = load_bias("b1", 2)
        b2p_sb = load_bias("b2p", 2)
        bd1_sb = load_bias("bd1", 4)
        bd2p_sb = load_bias("bd2p", 4)
        bv2p_sb = load_bias("bv2p", 2)

        ones1 = wpool.tile([1, P], F32, tag="ones1", name="ones1")
        nc.vector.memset(ones1, 1.0)
        ones32 = wpool.tile([NFC, 1], F32, tag="ones32", name="ones32")
        nc.vector.memset(ones32, 1.0)

        amat_sb = []
        for b in range(NSTRIP):
            a = wpool.tile([P, NFC], F32, tag=f"amat{b}", name=f"amat{b}")
            nc.sync.dma_start(out=a, in_=d["amat"][b])
            amat_sb.append(a)
        cvec_sb = wpool.tile([NFC, NQ], F32, tag="cvec", name="cvec")
        nc.sync.dma_start(out=cvec_sb, in_=d["cvec"])

        # ---- persistent feature tiles ----
        sh_u = [feat.tile([P, M], F32, tag=f"sh_u{i}", name=f"sh_u{i}") for i in range(2)]
        un_u = [feat.tile([P, M], F32, tag=f"un_u{i}", name=f"un_u{i}") for i in range(2)]
        v_u = [feat.tile([P, M], F32, tag=f"v_u{i}", name=f"v_u{i}") for i in range(6)]
        sh_n = [feat.tile([P, M], BF16, tag=f"sh_n{i}", name=f"sh_n{i}") for i in range(2)]
        un_n = [feat.tile([P, M], BF16, tag=f"un_n{i}", name=f"un_n{i}") for i in range(2)]
        v_n = [feat.tile([P, M], BF16, tag=f"v_n{i}", name=f"v_n{i}") for i in range(6)]

        # ---- encoder ----
        for c in range(NCH):
            cs = c * CH
            csl = slice(cs, cs + CH)

            def load_x(name, kt, tagp, shared_mod=None, vbufs=2):
                ts_ = []
                for k in range(kt):
                    tg = f"{tagp}{k % shared_mod}" if shared_mod else f"{tagp}{k}"
                    t = xin.tile([P, CH], F32, tag=tg, name=f"{tagp}{k}", bufs=vbufs)
                    nc.sync.dma_start(out=t, in_=d[name][k * P:(k + 1) * P, csl])
                    ts_.append(t)
                return ts_

            xs_t = load_x("x_s", 2, "xs")
            xl_t = load_x("x_l", 2, "xl")
            vs_t = load_x("v_s", 6, "vs", shared_mod=2, vbufs=3)
            vl_t = load_x("v_l", 6, "vl", shared_mod=2, vbufs=3)

            def layer(w_tiles, rhs_tiles, mt, evac):
                outs = []
                for m_ in range(mt):
                    pt = psp.tile([P, 512], F32, tag="ps", name="ps")
                    kt = len(rhs_tiles)
                    for k in range(kt):
                        nc.tensor.matmul(pt[:, 0:CH],
                                         w_tiles[k][:, m_ * P:(m_ + 1) * P],
                                         rhs_tiles[k],
                                         start=(k == 0), stop=(k == kt - 1))
                    outs.append(evac(m_, pt))
                return outs

            def act_evac(tag, bias_sb):
                # softplus(z+b) = ln(exp(z+b) + 1); exp & ln share one ACT table set
                def f(m_, pt):
                    e = xin.tile([P, CH], F32, tag=f"e{tag}{m_}", name=f"e{tag}{m_}", bufs=1)
                    if bias_sb is None:
                        nc.scalar.activation(e, pt[:, 0:CH], AF.Exp)
                    else:
                        nc.scalar.activation(e, pt[:, 0:CH], AF.Exp,
                                             bias=bias_sb[:, m_:m_ + 1])
                    t = xin.tile([P, CH], F32, tag=f"{tag}{m_}", name=f"{tag}{m_}", bufs=1)
                    nc.scalar.activation(t, e, AF.Ln, bias=1.0)
                    return t
                return f

            def add_evac(dst_tiles, bias_sb):
                def f(m_, pt):
                    nc.vector.tensor_scalar_add(dst_tiles[m_][:, csl], pt[:, 0:CH],
                                                bias_sb[:, m_:m_ + 1])
                    return None
                return f

            h1 = layer(w1c_t, xs_t + xl_t, 2, act_evac("h1", b1_sb))
            # s_inv with DVE evac into temp tiles
            s_ = []
            for m_ in range(2):
                pt = psp.tile([P, 512], F32, tag="ps", name="ps")
                for k in range(2):
                    nc.tensor.matmul(pt[:, 0:CH], w2_t[k][:, m_ * P:(m_ + 1) * P],
                                     h1[k], start=(k == 0), stop=(k == 1))
                t = xin.tile([P, CH], F32, tag=f"s{m_}", name=f"s{m_}", bufs=1)
                nc.vector.tensor_scalar_add(t, pt[:, 0:CH], b2p_sb[:, m_:m_ + 1])
                s_.append(t)
            hd = layer(wd1_t, s_, 4, act_evac("hd", bd1_sb))
            layer(wd2_t, hd, 4, add_evac(sh_u + un_u, bd2p_sb))
            for dd in range(3):
                rhs = [vs_t[2 * dd], vs_t[2 * dd + 1], vl_t[2 * dd], vl_t[2 * dd + 1]]
                v1 = layer(wv1c_t, rhs, 2, act_evac(f"v1_{dd}", None))
                layer(wv2_t, v1, 2, add_evac(v_u[2 * dd:2 * dd + 2], bv2p_sb))

        # ---- G1: unnormalized grams -> diag (per-node nsq) + off-diag sums ----
        if PH < 2:
            dbg = small.tile([1, 3], F32, tag="dbg", name="dbg")
            nc.vector.tensor_copy(dbg, sh_u[0][0:1, 0:3])
            nc.sync.dma_start(out=d_out, in_=dbg)
        if PH >= 2:
            ident3 = wpool.tile([P, 3 * P], F32, tag="ident3", name="ident3")
            nc.sync.dma_start(out=ident3, in_=d["ident3"])

            Q = small.tile([P, NSTRIP, NQ], F32, tag="Q", name="Q")
            nsqN = small.tile([P, NSTRIP, 5], F32, tag="nsqN", name="nsqN")

            pm3_sb = []
            for b in range(NSTRIP):
                t = wpool.tile([P, 3 * P], F32, tag=f"pm3_{b}", name=f"pm3_{b}")
                nc.sync.dma_start(out=t, in_=d["pm3"][b])
                pm3_sb.append(t)

            def gram(pt, tiles, ncol_tiles, b):
                bsl = slice(b * P, (b + 1) * P)
                for g in range(ncol_tiles):
                    for k in range(2):
                        nc.tensor.matmul(pt[:, g * P:(g + 1) * P],
                                         tiles[2 * g + k][:, bsl],
                                         tiles[2 * g + k][:, bsl],
                                         start=(k == 0), stop=(k == 1))
                return pt

            def msum(out_t, in0, in1, acc, width):
                # out = in0 * in1 ; acc[p] = row-sum(out)
                nc.vector.scalar_tensor_tensor(
                    out=out_t[:, 0:width], in0=in0, scalar=1.0, in1=in1,
                    op0=ALU.bypass, op1=ALU.mult, accum_out=acc)

            def sqsum(out_t, in_, acc, width):
                # out = in_^2 ; acc[p] = row-sum(out)
                nc.scalar.activation(out_t[:, 0:width], in_, AF.Square,
                                     accum_out=acc)

            for b in range(NSTRIP):
                gu_sh = psp.tile([P, 512], F32, tag="ps", name="gu_sh")
                gram(gu_sh, sh_u, 1, b)
                gu_un = psp.tile([P, 512], F32, tag="ps", name="gu_un")
                gram(gu_un, un_u, 1, b)
                gu_d = psp.tile([P, 512], F32, tag="ps", name="gu_d")
                gram(gu_d, v_u, 3, b)

                j5 = junkp.tile([P, P], F32, tag="jk1", name="j5")
                msum(j5, gu_sh[:, 0:P], pm3_sb[b][:, 0:P], Q[:, b, QSOFF:QSOFF + 1], P)
                j6 = junkp.tile([P, 3 * P], F32, tag="jk3", name="j6")
                msum(j6, gu_d[:, 0:3 * P], pm3_sb[b][:, 0:3 * P], Q[:, b, QVOFF:QVOFF + 1], 3 * P)
                jd0 = junkp.tile([P, P], F32, tag="jk1", name="jd0")
                msum(jd0, gu_sh[:, 0:P], ident3[:, 0:P], nsqN[:, b, 0:1], P)
                jd1 = junkp.tile([P, P], F32, tag="jk1", name="jd1")
                msum(jd1, gu_un[:, 0:P], ident3[:, 0:P], nsqN[:, b, 1:2], P)
                for dd in range(3):
                    jdv = junkp.tile([P, P], F32, tag="jk1", name=f"jdv{dd}")
                    msum(jdv, gu_d[:, dd * P:(dd + 1) * P], ident3[:, 0:P],
                         nsqN[:, b, 2 + dd:3 + dd], P)

            # Q nsq columns
            nc.vector.tensor_copy(Q[:, :, QNSQ_SH], nsqN[:, :, 0])
            nc.vector.tensor_copy(Q[:, :, QNSQ_UN], nsqN[:, :, 1])
            nc.vector.tensor_add(Q[:, :, QNSQ_V], nsqN[:, :, 2], nsqN[:, :, 3])
            nc.vector.tensor_add(Q[:, :, QNSQ_V], Q[:, :, QNSQ_V], nsqN[:, :, 4])

        if PH == 2:
            dbg = small.tile([1, 3], F32, tag="dbg", name="dbg")
            nc.vector.tensor_copy(dbg, Q[0:1, 0, 0:3])
            nc.sync.dma_start(out=d_out, in_=dbg)
        if PH >= 3:
            # ---- inverse norms (node-major) -> DRAM -> rows -> bcast -> normalize ----
            sq = small.tile([P, NSTRIP * 5], F32, tag="sqn", name="sqn")
            nc.vector.tensor_scalar_max(sq, nsqN.rearrange("p b s -> p (b s)"), 1e-24)
            nc.scalar.activation(sq, sq, AF.Ln)
            nc.scalar.activation(sq, sq, AF.Exp, scale=0.5)
            invN = small.tile([P, NSTRIP, 5], F32, tag="invN", name="invN")
            nc.vector.reciprocal(invN.rearrange("p b s -> p (b s)"), sq)
            for s in range(5):
                nc.sync.dma_start(out=d_inv[s:s + 1, :].rearrange("o (b p) -> p b o", p=P),
                                  in_=invN[:, :, s])

            sets_u = [sh_u, un_u, v_u[0:2], v_u[2:4], v_u[4:6]]
            sets_n = [sh_n, un_n, v_n[0:2], v_n[2:4], v_n[4:6]]
            for s in range(5):
                irow = rowp.tile([1, M], F32, tag="invrow", name="invrow")
                nc.sync.dma_start(out=irow, in_=d_inv[s:s + 1, :])
                for ci in range(NCH):
                    bc = psp.tile([P, 512], F32, tag="ps", name="bc")
                    nc.tensor.matmul(bc[:, 0:CH], ones1,
                                     irow[0:1, ci * CH:(ci + 1) * CH],
                                     start=True, stop=True)
                    for k in range(2):
                        nc.vector.tensor_mul(
                            sets_n[s][k][:, ci * CH:(ci + 1) * CH],
                            sets_u[s][k][:, ci * CH:(ci + 1) * CH],
                            bc[:, 0:CH])

        if PH == 3:
            dbg = small.tile([1, 3], F32, tag="dbg", name="dbg")
            nc.vector.tensor_copy(dbg, sh_n[0][0:1, 0:3])
            nc.sync.dma_start(out=d_out, in_=dbg)
        if PH >= 4:
            # ---- G2: normalized grams + masked pair-mse reductions ----
            for b in range(NSTRIP):
                g_sh = psp.tile([P, 512], F32, tag="ps", name="g_sh")
                gram(g_sh, sh_n, 1, b)
                g_un = psp.tile([P, 512], F32, tag="ps", name="g_un")
                gram(g_un, un_n, 1, b)
                g_d = psp.tile([P, 512], F32, tag="ps", name="g_d")
                gram(g_d, v_n, 3, b)

                tmb = maskp.tile([P, P], F32, tag="tm", name="tmb")
                nc.sync.dma_start(out=tmb, in_=d["tm"][b])

                spm = junkp.tile([P, P], F32, tag="spm", name="spm", bufs=2)
                msum(spm, g_sh[:, 0:P], pm3_sb[b][:, 0:P], Q[:, b, QB_SH:QB_SH + 1], P)
                j1 = junkp.tile([P, P], F32, tag="jk1", name="j1")
                sqsum(j1, spm[:, 0:P], Q[:, b, QA_SH:QA_SH + 1], P)

                upm = junkp.tile([P, P], F32, tag="upm", name="upm", bufs=2)
                msum(upm, g_un[:, 0:P], pm3_sb[b][:, 0:P], Q[:, b, QJUNK:QJUNK + 1], P)
                j2 = junkp.tile([P, P], F32, tag="jk1", name="j2")
                sqsum(j2, upm[:, 0:P], Q[:, b, QA_UN:QA_UN + 1], P)
                j3 = junkp.tile([P, P], F32, tag="jk1", name="j3")
                msum(j3, g_un[:, 0:P], tmb[:, 0:P], Q[:, b, QC_UN:QC_UN + 1], P)

                dpm = junkp.tile([P, 3 * P], F32, tag="dpm", name="dpm", bufs=2)
                msum(dpm, g_d[:, 0:3 * P], pm3_sb[b][:, 0:3 * P], Q[:, b, QB_DIR:QB_DIR + 1], 3 * P)
                j4 = junkp.tile([P, 3 * P], F32, tag="jk3", name="j4")
                sqsum(j4, dpm[:, 0:3 * P], Q[:, b, QA_DIR:QA_DIR + 1], 3 * P)

        if PH >= 4:
            # ---- segment reduction + final combine ----
            seg_ps = pseg.tile([NFC, NQ], F32, tag="seg", name="seg")
            for b in range(NSTRIP):
                nc.tensor.matmul(seg_ps, amat_sb[b], Q[:, b, :],
                                 start=(b == 0), stop=(b == NSTRIP - 1))
            segs = small.tile([NFC, NQ], F32, tag="segs", name="segs")
            nc.vector.tensor_copy(segs, seg_ps)

            acc = small.tile([NFC, 3], F32, tag="acc", name="acc")
            nc.vector.tensor_scalar_mul(acc[:, 0:1], segs[:, 0:1], cvec_sb[:, 0:1])
            for q in range(1, 11):
                nc.vector.scalar_tensor_tensor(
                    out=acc[:, 0:1], in0=segs[:, q:q + 1], scalar=cvec_sb[:, q:q + 1],
                    in1=acc[:, 0:1], op0=ALU.mult, op1=ALU.add)
            nc.vector.tensor_add(acc[:, 0:1], acc[:, 0:1], cvec_sb[:, QJUNK:QJUNK + 1])
            nc.vector.tensor_copy(acc[:, 1:2], segs[:, QNSQ_SH:QNSQ_SH + 1])
            nc.vector.tensor_copy(acc[:, 2:3], segs[:, QNSQ_UN:QNSQ_UN + 1])

            fin_ps = pseg.tile([1, 3], F32, tag="fin", name="fin")
            nc.tensor.matmul(fin_ps, ones32, acc, start=True, stop=True)
            fin_sb = small.tile([1, 3], F32, tag="fin_sb", name="fin_sb")
            nc.vector.tensor_copy(fin_sb, fin_ps)
            nc.sync.dma_start(out=d_out, in_=fin_sb)

    nc.compile()
    _NC_CACHE["nc"] = nc
    return nc


# ============================== entry point =================================

def kernel(**inputs) -> np.ndarray:
    from concourse.bass_utils import run_bass_kernel_spmd

    in_maps, n_valid = _shard_inputs(inputs)
    nc = build_nc()
    res = run_bass_kernel_spmd(nc, in_maps, core_ids=list(range(NCORES)))
    fins = [r["out"].reshape(3) for r in res.results]
    return _combine(fins, n_valid)



# revision 24
# speedup vs baseline: 2.9724x; 2.9724x over previous
"""Trainium2 Bass kernel for BalancedConformationalConsistencyLoss.

Strategy (segment/fragment parallelism, 8 cores):
  * Host sorts nodes by fragment; 32 fragments per core (snake-deal by size),
    bin-packed into 9 strips of 128 slots so no fragment straddles a strip.
  * Host also combines short/long inputs (x*R + y*(1-R)), fuses W2@Wd1 and
    folds all biases/ln2 shifts so the device runs a 2-matmul-deep scalar
    chain, all matmuls in bf16 (4x PE throughput vs fp32).
  * Device (per core, SPMD): feature-major encoder; per-strip 128x128 gram
    blocks on the tensor engine (unnormalized for variance + norms via the
    gram diagonal; normalized for pair-MSE terms); masked pair reductions
    via DVE/ACT/GPSIMD accumulators balanced across engines; one activation
    table (natural_log_exp_and_others) for Exp/Ln/Square - a post-compile
    pass removes the thrashing per-activation table reloads.
  * Host combines 8 partial losses (sum + the two global Frobenius sqrts).

All pairwise math uses the identity  sum_pm (S-t)^2 = A - 2tB + t^2*npairs
with A = sum pm*S^2, B = sum pm*S, so no (S-t) intermediates are built.
"""
import numpy as np
from contextlib import ExitStack

# ---------------- problem constants (hardcoded per contract) ----------------
N, D, NF = 8192, 256, 256
R = 0.3
BRICS = 0.4
AW = 0.6
VW = 0.2
CF = 0.1
NCORES = 8
NFC = NF // NCORES          # 32 fragments per core
P = 128                     # strip height
NSTRIP = 9                  # strips per core
M = NSTRIP * P              # 1152 padded slots per core
CH = 384                    # encoder column chunk
NCH = M // CH
LN2 = np.float32(np.log(2.0))

# Q column indices
(QB_SH, QA_SH, QA_UN, QC_UN, QB_DIR, QA_DIR,
 QSOFF, QVOFF, QNSQ_SH, QNSQ_UN, QNSQ_V, QJUNK) = range(12)
NQ = 12

ACT_TABLE_LN_EXP = "natural_log_exp_and_others"


# ============================ host-side prep ================================

def _assign_fragments(fid):
    counts = np.bincount(fid, minlength=NF)
    order = np.argsort(-counts, kind="stable")
    core_frags = [[] for _ in range(NCORES)]
    for i, f in enumerate(order):
        r = i // NCORES
        c = i % NCORES if r % 2 == 0 else NCORES - 1 - (i % NCORES)
        core_frags[c].append(int(f))
    layout = []
    for c in range(NCORES):
        frags = sorted(core_frags[c], key=lambda f: -counts[f])
        strips = [[] for _ in range(NSTRIP)]
        fill = np.zeros(NSTRIP, dtype=int)
        for f in frags:
            for s in range(NSTRIP):
                if fill[s] + counts[f] <= P:
                    strips[s].append(f)
                    fill[s] += counts[f]
                    break
            else:
                raise AssertionError(f"core {c}: fragment {f} does not fit")
        layout.append(strips)
    return layout, counts


def _build_core_meta(fid, atom_types, layout, counts):
    nodes_of = {f: np.nonzero(fid == f)[0] for f in range(NF)}
    metas = []
    for c in range(NCORES):
        slot_node = -np.ones(M, dtype=np.int64)
        slot_frag = -np.ones(M, dtype=np.int64)
        frag_global = []
        fl = 0
        for b in range(NSTRIP):
            pos = b * P
            for f in layout[c][b]:
                nn = nodes_of[f]
                slot_node[pos:pos + len(nn)] = nn
                slot_frag[pos:pos + len(nn)] = fl
                frag_global.append(f)
                pos += len(nn)
                fl += 1
        assert fl == NFC
        frag_global = np.array(frag_global, dtype=np.int64)
        real = slot_node >= 0

        pm3 = np.zeros((NSTRIP, P, 3 * P), dtype=np.float32)
        tm = np.zeros((NSTRIP, P, P), dtype=np.float32)
        amat = np.zeros((NSTRIP, P, NFC), dtype=np.float32)
        t2pm_frag = np.zeros(NFC, dtype=np.float32)
        for b in range(NSTRIP):
            sf = slot_frag[b * P:(b + 1) * P]
            sn = slot_node[b * P:(b + 1) * P]
            rr = sf >= 0
            same = (sf[:, None] == sf[None, :]) & rr[:, None] & rr[None, :]
            upper = np.triu(np.ones((P, P), dtype=bool), k=1)
            pmb = (same & upper).astype(np.float32)
            pm3[b] = np.tile(pmb, (1, 3))
            at = np.where(rr, atom_types[np.where(rr, sn, 0)], -1)
            tgt = np.where(at[:, None] == at[None, :], 0.3, 0.1).astype(np.float32)
            tm[b] = tgt * pmb
            for p in range(P):
                if rr[p]:
                    amat[b, p, sf[p]] = 1.0
            for fl_ in np.unique(sf[rr]):
                sel = sf == fl_
                t2pm_frag[fl_] += float(((tgt * pmb) ** 2)[np.ix_(sel, sel)].sum())

        cnt = counts[frag_global].astype(np.float32)
        valid = (cnt >= 2.0).astype(np.float32)
        pairs = cnt * (cnt - 1.0) * 0.5
        safe_c = np.maximum(cnt, 1.0)
        safe_p = np.maximum(pairs, 1.0)

        cvec = np.zeros((NFC, NQ), dtype=np.float32)
        t_sh = np.float32(BRICS)
        t_dir = np.float32(BRICS - 0.2)
        cvec[:, QB_SH] = -2.0 * t_sh * AW / safe_p
        cvec[:, QA_SH] = AW / safe_p
        cvec[:, QA_UN] = (1.0 - AW) / safe_p
        cvec[:, QC_UN] = -2.0 * (1.0 - AW) / safe_p
        cvec[:, QB_DIR] = -2.0 * t_dir * VW / (3.0 * safe_p)
        cvec[:, QA_DIR] = VW / (3.0 * safe_p)
        cvec[:, QSOFF] = -2.0 * AW / (safe_c * safe_c)
        cvec[:, QVOFF] = -2.0 * VW / (safe_c * safe_c)
        cvec[:, QNSQ_SH] = AW * (1.0 / safe_c - 1.0 / (safe_c * safe_c))
        cvec[:, QNSQ_V] = VW * (1.0 / safe_c - 1.0 / (safe_c * safe_c))
        cvec[:, QJUNK] = (AW * t_sh * t_sh * pairs / safe_p
                          + (1.0 - AW) * t2pm_frag / safe_p
                          + VW * t_dir * t_dir * pairs / safe_p)
        cvec *= valid[:, None]

        metas.append(dict(slot_node=slot_node, real=real, pm3=pm3, tm=tm,
                          amat=amat, cvec=cvec))
    return metas


def _shard_inputs(inputs):
    import ml_dtypes
    BF = ml_dtypes.bfloat16

    fid = np.asarray(inputs["fragment_ids"]).astype(np.int64)
    at = np.asarray(inputs["atom_types"]).astype(np.int64)
    layout, counts = _assign_fragments(fid)
    metas = _build_core_meta(fid, at, layout, counts)

    W1 = np.asarray(inputs["W1"], np.float32)
    W2 = np.asarray(inputs["W2"], np.float32)
    Wd1 = np.asarray(inputs["Wd1"], np.float32)
    Wd2 = np.asarray(inputs["Wd2"], np.float32)
    Wv1 = np.asarray(inputs["Wv1"], np.float32)
    Wv2 = np.asarray(inputs["Wv2"], np.float32)
    b1 = np.asarray(inputs["b1"], np.float32)
    b2 = np.asarray(inputs["b2"], np.float32)
    bd1 = np.asarray(inputs["bd1"], np.float32)
    bd2 = np.asarray(inputs["bd2"], np.float32)

    # fuse s_inv's linear layer into the decoder's first layer:
    #   hd_in = ssp(h1)@W2 + b2) @ Wd1 + bd1 = sp(h1) @ (W2@Wd1) + bd1p
    wf = np.ascontiguousarray(W2 @ Wd1, np.float32)
    bd1p = (bd1 + (b2 - LN2 * W2.sum(axis=0)) @ Wd1).astype(np.float32)
    bd2p = (bd2 - LN2 * Wd2.sum(axis=0)).astype(np.float32)
    bv2p = (-LN2 * Wv2.sum(axis=0)).astype(np.float32)

    # host-side combine of short/long branches
    xc = (np.asarray(inputs["scalar_short"], np.float32) * R
          + np.asarray(inputs["scalar_long"], np.float32) * (1.0 - R))
    vc = (np.asarray(inputs["vector_short"], np.float32) * R
          + np.asarray(inputs["vector_long"], np.float32) * (1.0 - R))

    ident = np.eye(P, dtype=np.float32)

    in_maps = []
    for c in range(NCORES):
        m = metas[c]
        idx = np.where(m["real"], m["slot_node"], 0)

        g2 = xc[idx] * m["real"][:, None]
        g3 = vc[idx] * m["real"][:, None, None]

        in_maps.append({
            "xc": np.ascontiguousarray(g2.T).astype(BF),
            "vc": np.ascontiguousarray(
                g3.transpose(1, 2, 0).reshape(3 * D, M)).astype(BF),
            "w1": W1.astype(BF), "wf": wf.astype(BF), "wd2": Wd2.astype(BF),
            "wv1": Wv1.astype(BF), "wv2": Wv2.astype(BF),
            "b1": b1.reshape(D, 1),
            "bd1p": bd1p.reshape(2 * D, 1),
            "bd2p": bd2p.reshape(2 * D, 1),
            "bv2p": bv2p.reshape(D, 1),
            "pm3": m["pm3"].astype(BF), "tm": m["tm"].astype(BF),
            "ident": ident.astype(BF),
            "amat": m["amat"], "cvec": m["cvec"],
        })
    n_valid = float((counts >= 2).sum())
    return in_maps, n_valid


def _combine(fins, n_valid):
    loss = float(sum(float(f[0]) for f in fins))
    ssq_sh = float(sum(float(f[1]) for f in fins))
    ssq_un = float(sum(float(f[2]) for f in fins))
    l2 = 0.01 * (np.sqrt(ssq_sh) + np.sqrt(ssq_un))
    if n_valid > 0:
        return np.float32(CF * (loss + n_valid * l2) / max(n_valid, 1.0))
    return np.float32(0.0)


# ============================ device program ================================

_NC_CACHE = {}


def _dedupe_act_table_loads(nc):
    """bacc's table placement alternates exp_and_others/natural_log, paying a
    1.3us ACT_TABLE_LOAD per softplus pass.  natural_log_exp_and_others holds
    Exp+Ln+Square together, so one load per block serves every activation."""
    from concourse.hw_specs import get_activation_tables
    tables = list(get_activation_tables(nc.m.arch).keys())
    want = tables.index(ACT_TABLE_LN_EXP)
    n_kept = n_dropped = 0
    for blk in nc.main_func.blocks:
        first = True
        kept = []
        for inst in blk.instructions:
            if type(inst).__name__ == "InstLoadActFuncSet":
                if first:
                    inst.act_func_set_id = want
                    first = False
                    n_kept += 1
                else:
                    n_dropped += 1
                    continue
            kept.append(inst)
        blk.instructions[:] = kept
    return n_kept, n_dropped


def build_nc():
    if "nc" in _NC_CACHE:
        return _NC_CACHE["nc"]
    import concourse.bass as bass
    import concourse.bacc as bacc
    import concourse.mybir as mybir
    import concourse.tile as tile

    F32 = mybir.dt.float32
    F16 = mybir.dt.float16
    BF16 = mybir.dt.bfloat16
    AF = mybir.ActivationFunctionType
    ALU = mybir.AluOpType

    nc = bacc.Bacc("TRN2", target_bir_lowering=False, debug=False)

    d = {}
    for name, shape, dt_ in [
        ("xc", [D, M], BF16), ("vc", [3 * D, M], BF16),
        ("w1", [D, D], BF16), ("wf", [D, 2 * D], BF16),
        ("wd2", [2 * D, 2 * D], BF16),
        ("wv1", [D, D], BF16), ("wv2", [D, D], BF16),
        ("b1", [D, 1], F32), ("bd1p", [2 * D, 1], F32),
        ("bd2p", [2 * D, 1], F32), ("bv2p", [D, 1], F32),
        ("pm3", [NSTRIP, P, 3 * P], BF16), ("tm", [NSTRIP, P, P], BF16),
        ("ident", [P, P], BF16),
        ("amat", [NSTRIP, P, NFC], F32), ("cvec", [NFC, NQ], F32),
    ]:
        d[name] = nc.dram_tensor(name, shape, dt_, kind="ExternalInput").ap()
    d_out = nc.dram_tensor("out", [1, 3], F32, kind="ExternalOutput").ap()
    d_inv = nc.dram_tensor("inv_scratch", [5, M], F16).ap()

    with tile.TileContext(nc) as tc, ExitStack() as ctx:
        wpool = ctx.enter_context(tc.tile_pool(name="w", bufs=1))
        feat = ctx.enter_context(tc.tile_pool(name="feat", bufs=1))
        xin = ctx.enter_context(tc.tile_pool(name="xin", bufs=2))
        small = ctx.enter_context(tc.tile_pool(name="small", bufs=1))
        rowp = ctx.enter_context(tc.tile_pool(name="rowp", bufs=2))
        junkp = ctx.enter_context(tc.tile_pool(name="junk", bufs=3))
        psp = ctx.enter_context(tc.tile_pool(name="ps", bufs=6, space="PSUM"))
        pseg = ctx.enter_context(tc.tile_pool(name="pseg", bufs=1, space="PSUM"))

        # ---- constants / weights ----
        def load_w(name, kt, cols):
            ts_ = []
            for k in range(kt):
                w = wpool.tile([P, cols], BF16, tag=f"{name}{k}", name=f"{name}{k}")
                nc.sync.dma_start(out=w, in_=d[name][k * P:(k + 1) * P, :])
                ts_.append(w)
            return ts_

        w1_t = load_w("w1", 2, D)
        wf_t = load_w("wf", 2, 2 * D)
        wd2_t = load_w("wd2", 4, 2 * D)
        wv1_t = load_w("wv1", 2, D)
        wv2_t = load_w("wv2", 2, D)

        def load_bias(name, mt):
            b = wpool.tile([P, mt], F32, tag=f"b_{name}", name=f"b_{name}")
            nc.sync.dma_start(out=b, in_=d[name].rearrange("(m p) o -> p (m o)", p=P))
            return b

        b1_sb = load_bias("b1", 2)
        bd1p_sb = load_bias("bd1p", 4)
        bd2p_sb = load_bias("bd2p", 4)
        bv2p_sb = load_bias("bv2p", 2)

        ones32 = wpool.tile([NFC, 1], F32, tag="ones32", name="ones32")
        nc.vector.memset(ones32, 1.0)
        ones1 = wpool.tile([1, P], F32, tag="ones1", name="ones1")
        nc.vector.memset(ones1, 1.0)

        # masks/segment maps ride the idle GPSIMD DMA queue so they don't
        # delay the feature loads on the sync queue
        ident_sb = wpool.tile([P, P], BF16, tag="ident", name="ident")
        nc.gpsimd.dma_start(out=ident_sb, in_=d["ident"])

        pm3_sb = []
        for b in range(NSTRIP):
            t = wpool.tile([P, 3 * P], BF16, tag=f"pm3_{b}", name=f"pm3_{b}")
            nc.gpsimd.dma_start(out=t, in_=d["pm3"][b])
            pm3_sb.append(t)
        tm_sb = []
        for b in range(NSTRIP):
            t = wpool.tile([P, P], BF16, tag=f"tm_{b}", name=f"tm_{b}")
            nc.gpsimd.dma_start(out=t, in_=d["tm"][b])
            tm_sb.append(t)
        amat_sb = []
        for b in range(NSTRIP):
            a = wpool.tile([P, NFC], F32, tag=f"amat{b}", name=f"amat{b}")
            nc.gpsimd.dma_start(out=a, in_=d["amat"][b])
            amat_sb.append(a)
        cvec_sb = wpool.tile([NFC, NQ], F32, tag="cvec", name="cvec")
        nc.gpsimd.dma_start(out=cvec_sb, in_=d["cvec"])

        # ---- persistent feature tiles (feature-major, bf16) ----
        sh_u = [feat.tile([P, M], BF16, tag=f"sh_u{i}", name=f"sh_u{i}") for i in range(2)]
        un_u = [feat.tile([P, M], BF16, tag=f"un_u{i}", name=f"un_u{i}") for i in range(2)]
        v_u = [feat.tile([P, M], BF16, tag=f"v_u{i}", name=f"v_u{i}") for i in range(6)]
        sh_n = [feat.tile([P, M], BF16, tag=f"sh_n{i}", name=f"sh_n{i}") for i in range(2)]
        un_n = [feat.tile([P, M], BF16, tag=f"un_n{i}", name=f"un_n{i}") for i in range(2)]
        v_n = [feat.tile([P, M], BF16, tag=f"v_n{i}", name=f"v_n{i}") for i in range(6)]
        bcs = [feat.tile([P, M], BF16, tag=f"bcs{i}", name=f"bcs{i}") for i in range(5)]

        Q = small.tile([P, NSTRIP, NQ], F32, tag="Q", name="Q")
        nsqN = small.tile([P, NSTRIP, 5], F32, tag="nsqN", name="nsqN")

        def gram(pt, tiles, col0, ncol_tiles, b):
            bsl = slice(b * P, (b + 1) * P)
            for g in range(ncol_tiles):
                for k in range(2):
                    nc.tensor.matmul(pt[:, (col0 + g) * P:(col0 + g + 1) * P],
                                     tiles[2 * g + k][:, bsl],
                                     tiles[2 * g + k][:, bsl],
                                     start=(k == 0), stop=(k == 1))
            return pt

        def msum(eng, out_t, in0, in1, acc, width):
            # out = in0 * in1 ; acc[p] = row-sum(out)
            eng.scalar_tensor_tensor(
                out=out_t[:, 0:width], in0=in0, scalar=1.0, in1=in1,
                op0=ALU.bypass, op1=ALU.mult, accum_out=acc)

        def g1_strip(b):
            # unnormalized grams -> diag (per-node nsq) + off-diag sums
            gA = psp.tile([P, 512], F32, tag="ps", name="gA")
            gram(gA, sh_u, 0, 1, b)
            gram(gA, un_u, 1, 1, b)
            gB = psp.tile([P, 512], F32, tag="ps", name="gB")
            gram(gB, v_u, 0, 3, b)
            for s, (pt, c0) in enumerate([(gA, 0), (gA, 1), (gB, 0), (gB, 1), (gB, 2)]):
                jd = junkp.tile([P, P], BF16, tag="jk1", name=f"jd{s}")
                msum(nc.vector, jd, pt[:, c0 * P:(c0 + 1) * P], ident_sb,
                     nsqN[:, b, s:s + 1], P)
            j5 = junkp.tile([P, P], BF16, tag="jk1", name="j5")
            msum(nc.vector, j5, gA[:, 0:P], pm3_sb[b][:, 0:P],
                 Q[:, b, QSOFF:QSOFF + 1], P)
            j6 = junkp.tile([P, 3 * P], BF16, tag="jk3", name="j6")
            msum(nc.vector, j6, gB[:, 0:3 * P], pm3_sb[b][:, 0:3 * P],
                 Q[:, b, QVOFF:QVOFF + 1], 3 * P)

        def g2_strip(b):
            # normalized grams + masked pair-mse reductions
            hA = psp.tile([P, 512], F32, tag="ps", name="hA")
            gram(hA, sh_n, 0, 1, b)
            gram(hA, un_n, 1, 1, b)
            hB = psp.tile([P, 512], F32, tag="ps", name="hB")
            gram(hB, v_n, 0, 3, b)

            def sqsum(src, acc, width, nm):
                # all-SBUF bf16 stt runs in the DVE 4x perf mode
                j = junkp.tile([P, 3 * P], BF16, tag="jsq", name=nm)
                nc.vector.scalar_tensor_tensor(
                    out=j[:, 0:width], in0=src, scalar=1.0, in1=src,
                    op0=ALU.bypass, op1=ALU.mult, accum_out=acc)

            spm = junkp.tile([P, P], BF16, tag="spm", name="spm", bufs=2)
            msum(nc.vector, spm, hA[:, 0:P], pm3_sb[b][:, 0:P],
                 Q[:, b, QB_SH:QB_SH + 1], P)
            sqsum(spm[:, 0:P], Q[:, b, QA_SH:QA_SH + 1], P, "j1")

            upm = junkp.tile([P, P], BF16, tag="upm", name="upm", bufs=2)
            msum(nc.vector, upm, hA[:, P:2 * P], pm3_sb[b][:, 0:P],
                 Q[:, b, QJUNK:QJUNK + 1], P)
            sqsum(upm[:, 0:P], Q[:, b, QA_UN:QA_UN + 1], P, "j2")
            j3 = junkp.tile([P, P], BF16, tag="jk1", name="j3")
            msum(nc.vector, j3, hA[:, P:2 * P], tm_sb[b][:, 0:P],
                 Q[:, b, QC_UN:QC_UN + 1], P)

            dpm = junkp.tile([P, 3 * P], BF16, tag="dpm", name="dpm", bufs=2)
            msum(nc.vector, dpm, hB[:, 0:3 * P], pm3_sb[b][:, 0:3 * P],
                 Q[:, b, QB_DIR:QB_DIR + 1], 3 * P)
            sqsum(dpm[:, 0:3 * P], Q[:, b, QA_DIR:QA_DIR + 1], 3 * P, "j4")

        # ---- encoder, G1 interleaved per chunk ----
        for c in range(NCH):
            cs = c * CH
            csl = slice(cs, cs + CH)

            def load_x(name, kt, tagp, shared_mod=None, vbufs=2):
                ts_ = []
                for k in range(kt):
                    tg = f"{tagp}{k % shared_mod}" if shared_mod else f"{tagp}{k}"
                    t = xin.tile([P, CH], BF16, tag=tg, name=f"{tagp}{k}", bufs=vbufs)
                    nc.sync.dma_start(out=t, in_=d[name][k * P:(k + 1) * P, csl])
                    ts_.append(t)
                return ts_

            xs_t = load_x("xc", 2, "xs")
            vs_t = load_x("vc", 6, "vs", shared_mod=2, vbufs=3)

            def softplus_layer(w_tiles, rhs_tiles, mt, tag, bias_sb):
                # out = softplus(w^T rhs + b) = ln(exp(z+b) + 1), bf16
                outs = []
                kt = len(rhs_tiles)
                for m_ in range(mt):
                    pt = psp.tile([P, 512], F32, tag="ps", name="ps")
                    for k in range(kt):
                        nc.tensor.matmul(pt[:, 0:CH],
                                         w_tiles[k][:, m_ * P:(m_ + 1) * P],
                                         rhs_tiles[k],
                                         start=(k == 0), stop=(k == kt - 1))
                    e = xin.tile([P, CH], F32, tag=f"e{tag}{m_}", name=f"e{tag}{m_}", bufs=1)
                    if bias_sb is None:
                        nc.scalar.activation(e, pt[:, 0:CH], AF.Exp)
                    else:
                        nc.scalar.activation(e, pt[:, 0:CH], AF.Exp,
                                             bias=bias_sb[:, m_:m_ + 1])
                    t = xin.tile([P, CH], BF16, tag=f"{tag}{m_}", name=f"{tag}{m_}", bufs=1)
                    nc.scalar.activation(t, e, AF.Ln, bias=1.0)
                    outs.append(t)
                return outs

            def linear_layer(w_tiles, rhs_tiles, mt, dst_tiles, bias_sb, on_act):
                kt = len(rhs_tiles)
                for m_ in range(mt):
                    pt = psp.tile([P, 512], F32, tag="ps", name="ps")
                    for k in range(kt):
                        nc.tensor.matmul(pt[:, 0:CH],
                                         w_tiles[k][:, m_ * P:(m_ + 1) * P],
                                         rhs_tiles[k],
                                         start=(k == 0), stop=(k == kt - 1))
                    if on_act:
                        nc.scalar.activation(dst_tiles[m_][:, csl], pt[:, 0:CH],
                                             AF.Identity,
                                             bias=bias_sb[:, m_:m_ + 1])
                    else:
                        nc.vector.tensor_scalar_add(dst_tiles[m_][:, csl],
                                                    pt[:, 0:CH],
                                                    bias_sb[:, m_:m_ + 1])

            h1 = softplus_layer(w1_t, xs_t, 2, "h1", b1_sb)
            hd = softplus_layer(wf_t, h1, 4, "hd", bd1p_sb)
            linear_layer(wd2_t, hd, 4, sh_u + un_u, bd2p_sb, on_act=False)
            for dd in range(3):
                v1 = softplus_layer(wv1_t, vs_t[2 * dd:2 * dd + 2], 2, f"v1_{dd}", None)
                linear_layer(wv2_t, v1, 2, v_u[2 * dd:2 * dd + 2], bv2p_sb, on_act=True)
            for b in range(3 * c, 3 * c + 3):
                g1_strip(b)

        # Q nsq columns
        nc.vector.tensor_copy(Q[:, :, QNSQ_SH], nsqN[:, :, 0])
        nc.vector.tensor_copy(Q[:, :, QNSQ_UN], nsqN[:, :, 1])
        nc.vector.tensor_add(Q[:, :, QNSQ_V], nsqN[:, :, 2], nsqN[:, :, 3])
        nc.vector.tensor_add(Q[:, :, QNSQ_V], Q[:, :, QNSQ_V], nsqN[:, :, 4])

        # ---- inverse norms -> DRAM bounce -> broadcast (GPSIMD) ----
        # floor keeps inv finite in fp16 (padded slots have nsq == 0)
        sq = small.tile([P, NSTRIP * 5], F32, tag="sqn", name="sqn")
        nc.vector.tensor_scalar_max(sq, nsqN.rearrange("p b s -> p (b s)"), 1e-6)
        nc.scalar.activation(sq, sq, AF.Ln)
        nc.scalar.activation(sq, sq, AF.Exp, scale=0.5)
        invN = small.tile([P, NSTRIP, 5], F32, tag="invN", name="invN")
        nc.vector.reciprocal(invN.rearrange("p b s -> p (b s)"), sq)
        invH = small.tile([P, NSTRIP, 5], F16, tag="invH", name="invH")
        nc.vector.tensor_copy(invH.rearrange("p b s -> p (b s)"),
                              invN.rearrange("p b s -> p (b s)"))
        for s in range(5):
            nc.sync.dma_start(out=d_inv[s:s + 1, :].rearrange("o (b p) -> p b o", p=P),
                              in_=invH[:, :, s])

        sets_u = [sh_u, un_u, v_u[0:2], v_u[2:4], v_u[4:6]]
        sets_n = [sh_n, un_n, v_n[0:2], v_n[2:4], v_n[4:6]]
        irows = []
        for s in range(5):
            irow = rowp.tile([1, M], F16, tag=f"invrow{s}", name=f"invrow{s}")
            nc.sync.dma_start(out=irow, in_=d_inv[s:s + 1, :])
            irows.append(irow)

        # ---- normalize + G2, interleaved per chunk ----
        for ci in range(NCH):
            cslc = slice(ci * CH, (ci + 1) * CH)
            for s in range(5):
                nc.gpsimd.partition_broadcast(bcs[s][:, cslc], irows[s][0:1, cslc])
                for k in range(2):
                    nc.vector.tensor_mul(
                        sets_n[s][k][:, cslc], sets_u[s][k][:, cslc], bcs[s][:, cslc])
            for b in range(3 * ci, 3 * ci + 3):
                g2_strip(b)

        # ---- segment reduction + final combine ----
        seg_ps = pseg.tile([NFC, NQ], F32, tag="seg", name="seg")
        for b in range(NSTRIP):
            nc.tensor.matmul(seg_ps, amat_sb[b], Q[:, b, :],
                             start=(b == 0), stop=(b == NSTRIP - 1))
        segs = small.tile([NFC, NQ], F32, tag="segs", name="segs")
        nc.vector.tensor_copy(segs, seg_ps)

        acc = small.tile([NFC, 3], F32, tag="acc", name="acc")
        prod = small.tile([NFC, 11], F32, tag="prod", name="prod")
        nc.vector.tensor_mul(prod, segs[:, 0:11], cvec_sb[:, 0:11])
        nc.vector.tensor_reduce(acc[:, 0:1], prod, mybir.AxisListType.X, ALU.add)
        nc.vector.tensor_add(acc[:, 0:1], acc[:, 0:1], cvec_sb[:, QJUNK:QJUNK + 1])
        nc.vector.tensor_copy(acc[:, 1:2], segs[:, QNSQ_SH:QNSQ_SH + 1])
        nc.vector.tensor_copy(acc[:, 2:3], segs[:, QNSQ_UN:QNSQ_UN + 1])

        fin_ps = pseg.tile([1, 3], F32, tag="fin", name="fin")
        nc.tensor.matmul(fin_ps, ones32, acc, start=True, stop=True)
        fin_sb = small.tile([1, 3], F32, tag="fin_sb", name="fin_sb")
        nc.vector.tensor_copy(fin_sb, fin_ps)
        nc.sync.dma_start(out=d_out, in_=fin_sb)

    nc.compile()
    _dedupe_act_table_loads(nc)
    _NC_CACHE["nc"] = nc
    return nc


# ============================== entry point =================================

def kernel(**inputs) -> np.ndarray:
    from concourse.bass_utils import run_bass_kernel_spmd

    in_maps, n_valid = _shard_inputs(inputs)
    nc = build_nc()
    res = run_bass_kernel_spmd(nc, in_maps, core_ids=list(range(NCORES)))
    fins = [r["out"].reshape(3) for r in res.results]
    return _combine(fins, n_valid)
